# revision 40
# baseline (speedup 1.0000x reference)
"""AnyVariateAttention Trainium2 kernel (8 NeuronCores, SPMD).

Sharding: 16 (batch, head) pairs / 8 cores -> core c computes 2 adjacent heads
of batch c//4 (heads 2*(c%4), 2*(c%4)+1).

Host precomputes QKV projections + partial RoPE (cheap O(N*D^2) work) and the
final output projection; the device runs only the O(N^2) attention part.

v3: fp8 DoubleRow score matmuls, bias folded into the matmul, per-engine
PSUM rings, greedy exp routing.  159.9us -> 155.7us (cost-model timeline).

- scores: fp8e4m3 DoubleRow matmuls at 0.5 PE-cycles/row.  Precision comes
  from a hi/lo split: 128 product rows [q_h*k_h | (q_h/4)*(4*k_l) |
  (8*q_l)*(k_h/8) | (8*q_l)*(k_l/8)] + 1 bias row (k side = 1.0, q side =
  the per-(head,class) attention bias) + 1 zero pad = 130 rows = 65
  partitions x 2 DoubleRow slices.  Two q variants carry the same-variate /
  cross-variate bias; the matmul for chunk c of q-tile t picks the variant.
  End-to-end rel err ~6e-3 (vs 2e-2 budget).
- exp is the wall: every score element must leave PSUM through ACT or DVE
  (GPSIMD cannot access PSUM, DMA cannot read PSUM) at 1 elem/lane/cycle.
  With the bias folded into PSUM, exp instructions need no per-class bias
  column, so tiles can group ARBITRARY chunks.  PSUM rings per engine:
  ACT 2x[128,1024], DVE alternating [128,1024]+[128,512], PV [128,264]
  = 15.2KB of the 16KB partition budget.  A greedy list scheduler assigns
  each chunk-group to whichever engine frees up first.
- PV: q in PSUM partitions, out free dim = 33 (head-dim 32 + ones column
  for the softmax denominator), accumulated over 32 k-chunks per q-tile.
- out: unnormalized [pv|den] copied PSUM->SBUF on the less-loaded engine
  and DMAd to DRAM; the host divides by the denominator and applies the
  output projection.
"""

import sys
import numpy as np

for _p in ("/opt/trn_rl_repo",):
    if _p not in sys.path:
        sys.path.insert(0, _p)

import ml_dtypes

BF16 = ml_dtypes.bfloat16
FP8 = ml_dtypes.float8_e4m3

B, N, D, H, HD = 2, 4096, 256, 8, 32
SEQ = 512
SCALE = HD ** -0.5
NCORES = 8
SCHRAUD_A = 184.6650390625   # 128 * log2(e)
SCHRAUD_B0 = 16256.0
SCHRAUD_ADJ = -7.4

# effective engine times (ns) for greedy routing
ACT_T1024 = (1024 + 222) / 1.2
DVE_T1024 = (1024 + 120) / 0.96
DVE_T512 = (512 + 120) / 0.96

_NC_CACHE = {}


TD_SKEW = 250.0
K_FIRST = 8
D_END_BONUS = 0.0
FIRST_SMALL = 2
FORCE_D_AT = -1
D_PARITY0 = 1


EXP_LAG = 1
PV_LAG = 4


def _build_nc(stage=6):
    import concourse.bass as bass  # noqa: F401
    import concourse.tile as tile
    from concourse import bacc, mybir

    from concourse.alu_op_type import AluOpType
    bf = mybir.dt.bfloat16
    f32 = mybir.dt.float32
    i16 = mybir.dt.int16
    fp8 = mybir.dt.float8e4
    EXP = mybir.ActivationFunctionType.Exp
    DR = mybir.MatmulPerfMode.DoubleRow

    nc = bacc.Bacc("TRN2", target_bir_lowering=False, debug=False,
                   num_devices=NCORES)

    # q: [65, (j2, t8, var2, i2, 512)]  k: [65, (j2, c32, i2, 128)]
    q_d = nc.declare_dram_parameter("q", [65, 32768], fp8, isOutput=False)
    k_d = nc.declare_dram_parameter("k", [65, 16384], fp8, isOutput=False)
    v_d = nc.declare_dram_parameter("v", [128, 32 * 2 * 33], bf, isOutput=False)
    out_d = nc.declare_dram_parameter("out", [128, 8 * 264], f32, isOutput=True)

    NT = N // 512        # 8 q-tiles of 512
    NCP = 16             # 16 chunk-pairs of 2x128 k rows per (h, t)

    # step order: for t, for h, for p; chunks stream 2 per step
    steps = [(t, h, p) for t in range(NT) for h in range(2) for p in range(NCP)]
    n_steps = len(steps)
    # chunk stream: global chunk g = 2*s + j covers (t, h, c=2p+j)
    n_chunks = 2 * n_steps

    def chunk_info(g):
        t, h, p = steps[g // 2]
        c = 2 * p + (g % 2)
        same = (c // 4 == t)
        return t, h, c, same

    # --- greedy exp-tile schedule over the 128-col quarter stream ---------
    # units (in quarters of 128 cols): ACT tile = 8 (spa, ring-2);
    # DVE alternates 9 (spd1 [1152]) and 4 (spd2 [512]).
    # tiles[i] = (engine, pool_id, q0, nq)
    n_q = 4 * n_chunks
    tiles = []
    ta, td = 0.0, TD_SKEW  # startup skew: DVE's first tile lands later
    d_parity = D_PARITY0
    qq = 0
    COPY_A = (264 + 222) / 1.2
    COPY_D = (264 + 120) / 0.96
    copy_eng = []          # engine per out-copy (t order)
    next_copy_q = 256      # after t=0's quarters (64 chunks * 4 per t)

    def a_cost_of(w):
        return (w + 222) / 1.2

    def d_cost_of(w):
        return (w + 120) / 0.96

    while qq < n_q:
        if qq >= next_copy_q:
            if ta + COPY_A <= td + COPY_D:
                copy_eng.append(0)
                ta += COPY_A
            else:
                copy_eng.append(1)
                td += COPY_D
            next_copy_q += 256
        # first few units are half-size so each engine's first exp can
        # start as soon as a single chunk's scores land
        a_nq = min(4 if len(tiles) < FIRST_SMALL else 8, n_q - qq)
        d_nq = min((4 if len(tiles) < FIRST_SMALL else 8)
                   if d_parity == 0 else 4, n_q - qq)
        d_pool = 1 if d_parity == 0 else 2
        force_d = (len(tiles) == FORCE_D_AT)
        end_bonus = D_END_BONUS if qq > n_q - 420 else 0.0
        if not force_d and \
                ta + a_cost_of(128 * a_nq) <= \
                td + d_cost_of(128 * d_nq) - end_bonus:
            tiles.append(("A", 0, qq, a_nq))
            ta += a_cost_of(128 * a_nq)
            qq += a_nq
        else:
            tiles.append(("D", d_pool, qq, d_nq))
            td += d_cost_of(128 * d_nq)
            d_parity ^= 1
            qq += d_nq
    copy_eng.append(0 if ta <= td else 1)  # final t's copy

    # map: quarter -> (tile_idx, offset_in_tile_in_quarters)
    quarter_loc = {}
    for ti, (_, _, q0, nq) in enumerate(tiles):
        for o in range(nq):
            quarter_loc[q0 + o] = (ti, o)
    # tile of the last quarter of step s (exp(s) ready once this tile done)
    tile_of_step = [quarter_loc[8 * s + 7][0] for s in range(n_steps)]

    with tile.TileContext(nc) as tc:
        from contextlib import ExitStack

        with ExitStack() as ctx:
            const = ctx.enter_context(tc.tile_pool(name="const", bufs=1))

            # dim1 = (j*8 + t)*2 + var  /  j*32 + c
            q_sb = const.tile([65, 32, 2, 512], fp8, tag="q_sb")
            k_sb = const.tile([65, 64, 2, 128], fp8, tag="k_sb")
            v_sb = const.tile([128, 32 * 2 * 33], bf, tag="v_sb")

            def q_ap(j, t, var):
                return q_sb[:, (j * 8 + t) * 2 + var]

            # staged input DMAs: first tiles' operands land early.
            # chunks 0-3 of (h0,t0) are same-class -> var0 slice first.
            kf = K_FIRST
            nc.sync.dma_start(k_sb[:, 0:kf], k_d[:, 0:256 * kf])   # h0 first
            nc.sync.dma_start(q_sb[:, 0:1], q_d[:, 0:1024])        # h0 t0 var0
            nc.sync.dma_start(q_sb[:, 1:2], q_d[:, 1024:2048])     # h0 t0 var1
            nc.sync.dma_start(k_sb[:, kf:32], k_d[:, 256 * kf:8192])  # h0 rest
            nc.sync.dma_start(v_sb[:, 0:528], v_d[:, 0:528])
            nc.sync.dma_start(k_sb[:, 32:64], k_d[:, 8192:16384])  # h1
            nc.sync.dma_start(q_sb[:, 16:18], q_d[:, 16384:18432])  # h1 t0
            nc.sync.dma_start(v_sb[:, 528:2112], v_d[:, 528:2112])
            nc.sync.dma_start(q_sb[:, 2:16], q_d[:, 2048:16384])
            nc.sync.dma_start(q_sb[:, 18:32], q_d[:, 18432:32768])

            # PSUM: ACT ring 2x[1024] + DVE [1024]+[512] + PV [264]
            spa = ctx.enter_context(
                tc.tile_pool(name="spa", bufs=2, space="PSUM"))
            spd1 = ctx.enter_context(
                tc.tile_pool(name="spd1", bufs=1, space="PSUM"))
            spd2 = ctx.enter_context(
                tc.tile_pool(name="spd2", bufs=1, space="PSUM"))
            pvp = ctx.enter_context(
                tc.tile_pool(name="pvp", bufs=1, space="PSUM"))
            ptpa = ctx.enter_context(tc.tile_pool(name="ptpa", bufs=5))
            ptp1 = ctx.enter_context(tc.tile_pool(name="ptp1", bufs=4))
            ptp2 = ctx.enter_context(tc.tile_pool(name="ptp2", bufs=3))
            osp = ctx.enter_context(tc.tile_pool(name="osp", bufs=2))

            sp_tiles = {}   # tile_idx -> psum tile
            pt_tiles = {}   # tile_idx -> pt AP (bf16 view)
            pv_tiles = {}   # t -> pv psum tile

            def emit_scores_tile(ti):
                eng, pool_id, q0, nq = tiles[ti]
                w = 128 * nq
                pool = spa if pool_id == 0 else (spd1 if pool_id == 1 else spd2)
                wal = 128 * nq if pool_id == 0 else (1024 if pool_id == 1 else 512)
                sp = pool.tile([128, wal], f32, tag=f"sp{pool_id}",
                               name=f"sp{ti}")
                sp_tiles[ti] = sp
                # one matmul per contiguous quarter-run within a chunk
                q = q0
                while q < q0 + nq:
                    ch = q // 4
                    qe = min((ch + 1) * 4, q0 + nq)
                    cnt = qe - q
                    t, h, c, same = chunk_info(ch)
                    var = 0 if same else 1
                    qc0 = q % 4
                    nc.tensor.matmul(
                        sp[:, (q - q0) * 128:(qe - q0) * 128],
                        lhsT=k_sb[:, h * 32 + c],
                        rhs=q_ap(h, t, var)[:, :, qc0 * 128:
                                            (qc0 + cnt) * 128],
                        start=True, stop=True, perf_mode=DR)
                    q = qe

            def emit_exp_tile(ti):
                eng, pool_id, q0, nq = tiles[ti]
                w = 128 * nq
                sp = sp_tiles.pop(ti)
                if eng == "A":
                    pt = ptpa.tile([128, 1024], bf, tag="pt", name=f"pt{ti}")
                    nc.scalar.activation(
                        pt[:, 0:w], sp[:, 0:w], EXP, bias=0.0, scale=1.0)
                    pt_tiles[ti] = pt[:]
                else:
                    pool, wal = (ptp1, 1024) if pool_id == 1 else (ptp2, 512)
                    pt = pool.tile([128, wal], i16, tag=f"pti{pool_id}",
                                   name=f"pte{ti}")
                    nc.vector.tensor_scalar(
                        pt[:, 0:w], sp[:, 0:w], SCHRAUD_A,
                        SCHRAUD_B0 + SCHRAUD_ADJ,
                        AluOpType.mult, AluOpType.add)
                    pt_tiles[ti] = pt[:].bitcast(bf)

            def emit_pv_step(s):
                t, h, p = steps[s]
                if h == 0 and p == 0:
                    pv_tiles[t] = pvp.tile([128, 264], f32, tag="pv",
                                           name=f"pv{t}")
                pv = pv_tiles[t]
                for j in range(2):
                    c = 2 * p + j
                    for qc in range(4):
                        ti, o = quarter_loc[8 * s + 4 * j + qc]
                        src = pt_tiles[ti]
                        first = (h == 0 and c == 0 and qc == 0)
                        nc.tensor.matmul(
                            pv[:, (h * 4 + qc) * 33:(h * 4 + qc + 1) * 33],
                            lhsT=src[:, o * 128:(o + 1) * 128],
                            rhs=v_sb[:, (c * 2 + h) * 33:(c * 2 + h + 1) * 33],
                            start=first, stop=(c == 31),
                            skip_group_check=True)

            def emit_out(t):
                pv = pv_tiles.pop(t)
                ot = osp.tile([128, 264], f32, tag="ot", name=f"ot{t}")
                if copy_eng[t] == 0:
                    nc.scalar.copy(ot[:], pv[:])
                else:
                    nc.vector.tensor_copy(ot[:], pv[:])
                nc.sync.dma_start(out_d[:, t * 264:(t + 1) * 264], ot[:])

            # software pipeline over steps: scores stream per tile; exp fires
            # one step after a tile's last chunk; PV lags 4 steps.
            next_tile = 0        # next score tile to emit
            exp_done = -1        # last exp-emitted tile
            for s in range(n_steps + 6):
                # emit score tiles covering chunks of step s
                while next_tile < len(tiles) and \
                        tiles[next_tile][2] <= 8 * s + 7 and s < n_steps:
                    emit_scores_tile(next_tile)
                    next_tile += 1
                if 0 <= s - PV_LAG < n_steps:
                    emit_pv_step(s - PV_LAG)
                    # free pt tiles fully consumed (all chunks of tiles
                    # belonging to steps <= s-4 and not needed later)
                if 0 <= s - EXP_LAG < n_steps:
                    # exp for all tiles completed by step s-EXP_LAG
                    target = tile_of_step[s - EXP_LAG]
                    while exp_done < target:
                        exp_done += 1
                        emit_exp_tile(exp_done)
                so = s - 5
                if 0 <= so < n_steps:
                    t, h, p = steps[so]
                    if h == 1 and p == NCP - 1:
                        emit_out(t)

    nc.compile()
    return nc


def _rope(x, positions):
    # x: [..., N, hd]; partial RoPE (rope_percent=0.5)
    half = HD // 2
    ra = half // 2
    frac = 2.0 * np.arange(ra, dtype=np.float32) / HD
    ts = (10000.0 ** frac).astype(np.float32)
    sinu = positions[:, None] / ts[None, :]
    sin = np.sin(sinu).astype(np.float32)
    cos = np.cos(sinu).astype(np.float32)
    f, s = x[..., :half], x[..., half:]
    fr, fp = f[..., :ra], f[..., ra:]
    sr, sp = s[..., :ra], s[..., ra:]
    return np.concatenate(
        [fr * cos - sr * sin, fp, sr * cos + fr * sin, sp], axis=-1)


def _fp8(x):
    return np.asarray(x, dtype=np.float32).astype(FP8)


def kernel(**inputs):
    hs = np.asarray(inputs["hidden_states"], dtype=np.float32)
    qw = np.asarray(inputs["q_w"], dtype=np.float32)
    kw = np.asarray(inputs["k_w"], dtype=np.float32)
    vw = np.asarray(inputs["v_w"], dtype=np.float32)
    ow = np.asarray(inputs["o_w"], dtype=np.float32)
    obb = np.asarray(inputs["o_b"], dtype=np.float32)
    qb_ = np.asarray(inputs["q_b"], dtype=np.float32)
    kb_ = np.asarray(inputs["k_b"], dtype=np.float32)
    vb_ = np.asarray(inputs["v_b"], dtype=np.float32)
    ab = np.asarray(inputs["attention_biases"], dtype=np.float32)
    seq = int(np.asarray(inputs["sequence_length"]))
    assert seq == SEQ, f"kernel compiled for sequence_length={SEQ}, got {seq}"
    assert hs.shape == (B, N, D)

    if ("nc", 6) not in _NC_CACHE:
        _NC_CACHE[("nc", 6)] = _build_nc(6)
    nc = _NC_CACHE[("nc", 6)]

    # host-side projections + rope (f32)
    pos = np.arange(N, dtype=np.float32)
    q = (hs @ qw.T + qb_) * SCALE    # [B, N, D]
    k = hs @ kw.T + kb_
    v = hs @ vw.T + vb_
    q = q.reshape(B, N, H, HD).transpose(0, 2, 1, 3)  # [B, H, N, hd]
    k = k.reshape(B, N, H, HD).transpose(0, 2, 1, 3)
    v = v.reshape(B, N, H, HD).transpose(0, 2, 1, 3)
    q = _rope(q, pos)
    k = _rope(k, pos)

    # fp8 hi/lo factor arrays (shared across cores)
    QH = _fp8(q)
    QHf = QH.astype(np.float32)
    QL8 = _fp8((q - QHf) * 8.0)
    QH4 = _fp8(QHf / 4.0)
    KH = _fp8(k)
    KHf = KH.astype(np.float32)
    KL4 = _fp8((k - KHf) * 4.0)
    KH8 = _fp8(KHf / 8.0)
    KL32 = _fp8(KL4.astype(np.float32) / 32.0)

    in_maps = []
    for c in range(NCORES):
        b = c // 4
        h0 = 2 * (c % 4)
        # q: [65, j, t, var, i, 512]; slice0 rows = [QH(32); QH4(32); bias],
        # slice1 rows = [QL8(32); QL8(32); 0]
        q_t = np.zeros((65, 2, 8, 2, 2, 512), dtype=FP8)
        k_t = np.zeros((65, 2, 32, 2, 128), dtype=FP8)
        v_t = np.empty((128, 32, 2, 33), dtype=np.float32)
        for j in range(2):
            h = h0 + j
            qh = QH[b, h].reshape(8, 512, HD)    # [t, col, hd]
            qh4 = QH4[b, h].reshape(8, 512, HD)
            ql8 = QL8[b, h].reshape(8, 512, HD)
            for var in range(2):
                q_t[0:32, j, :, var, 0] = qh.transpose(2, 0, 1)
                q_t[32:64, j, :, var, 0] = qh4.transpose(2, 0, 1)
                q_t[0:32, j, :, var, 1] = ql8.transpose(2, 0, 1)
                q_t[32:64, j, :, var, 1] = ql8.transpose(2, 0, 1)
                q_t[64, j, :, var, 0] = np.float32(ab[h, var]).astype(FP8)
            kh = KH[b, h].reshape(32, 128, HD)   # [c, col, hd]
            kl4 = KL4[b, h].reshape(32, 128, HD)
            kh8 = KH8[b, h].reshape(32, 128, HD)
            kl32 = KL32[b, h].reshape(32, 128, HD)
            k_t[0:32, j, :, 0] = kh.transpose(2, 0, 1)
            k_t[32:64, j, :, 0] = kl4.transpose(2, 0, 1)
            k_t[64, j, :, 0] = 1.0
            k_t[0:32, j, :, 1] = kh8.transpose(2, 0, 1)
            k_t[32:64, j, :, 1] = kl32.transpose(2, 0, 1)
            v_t[:, :, j, :32] = v[b, h].reshape(32, 128, 32).transpose(1, 0, 2)
            v_t[:, :, j, 32] = 1.0
        in_maps.append({
            "q": np.ascontiguousarray(q_t.reshape(65, 32768)),
            "k": np.ascontiguousarray(k_t.reshape(65, 16384)),
            "v": np.ascontiguousarray(
                v_t.reshape(128, 32 * 2 * 33)).astype(BF16),
        })

    global _LAST_IN_MAPS, _LAST_RESULTS
    _LAST_IN_MAPS = in_maps
    from concourse.bass_utils import run_bass_kernel_spmd
    res = run_bass_kernel_spmd(nc, in_maps, core_ids=list(range(NCORES)))
    _LAST_RESULTS = res.results

    attn = np.empty((B, H, N, HD), dtype=np.float32)
    for c in range(NCORES):
        b = c // 4
        h0 = 2 * (c % 4)
        o = res.results[c]["out"].reshape(128, 8, 2, 4, 33)
        for j in range(2):
            # q = 512*t + 128*qc + row
            pv = o[:, :, j, :, :32]    # [row, t, qc, 32]
            den = o[:, :, j, :, 32]    # [row, t, qc]
            x = pv / den[..., None]
            attn[b, h0 + j] = x.transpose(1, 2, 0, 3).reshape(N, HD)

    ctx = attn.transpose(0, 2, 1, 3).reshape(B, N, D)
    return ctx @ ow.T + obb[None, None, :]


# revision 45
# speedup vs baseline: 1.0012x; 1.0012x over previous
"""AnyVariateAttention Trainium2 kernel (8 NeuronCores, SPMD).

Sharding: 16 (batch, head) pairs / 8 cores -> core c computes 2 adjacent heads
of batch c//4 (heads 2*(c%4), 2*(c%4)+1).

Host precomputes QKV projections + partial RoPE (cheap O(N*D^2) work) and the
final output projection; the device runs only the O(N^2) attention part.

v3: fp8 DoubleRow score matmuls, bias folded into the matmul, per-engine
PSUM rings, greedy exp routing.  159.9us -> 155.7us (cost-model timeline).

- scores: fp8e4m3 DoubleRow matmuls at 0.5 PE-cycles/row.  Precision comes
  from a hi/lo split: 128 product rows [q_h*k_h | (q_h/4)*(4*k_l) |
  (8*q_l)*(k_h/8) | (8*q_l)*(k_l/8)] + 1 bias row (k side = 1.0, q side =
  the per-(head,class) attention bias) + 1 zero pad = 130 rows = 65
  partitions x 2 DoubleRow slices.  Two q variants carry the same-variate /
  cross-variate bias; the matmul for chunk c of q-tile t picks the variant.
  End-to-end rel err ~6e-3 (vs 2e-2 budget).
- exp is the wall: every score element must leave PSUM through ACT or DVE
  (GPSIMD cannot access PSUM, DMA cannot read PSUM) at 1 elem/lane/cycle.
  With the bias folded into PSUM, exp instructions need no per-class bias
  column, so tiles can group ARBITRARY chunks.  PSUM rings per engine:
  ACT 2x[128,1024], DVE alternating [128,1024]+[128,512], PV [128,264]
  = 15.2KB of the 16KB partition budget.  A greedy list scheduler assigns
  each chunk-group to whichever engine frees up first.
- PV: q in PSUM partitions, out free dim = 33 (head-dim 32 + ones column
  for the softmax denominator), accumulated over 32 k-chunks per q-tile.
- out: unnormalized [pv|den] copied PSUM->SBUF on the less-loaded engine
  and DMAd to DRAM; the host divides by the denominator and applies the
  output projection.
"""

import sys
import numpy as np

for _p in ("/opt/trn_rl_repo",):
    if _p not in sys.path:
        sys.path.insert(0, _p)

import ml_dtypes

BF16 = ml_dtypes.bfloat16
FP8 = ml_dtypes.float8_e4m3

B, N, D, H, HD = 2, 4096, 256, 8, 32
SEQ = 512
SCALE = HD ** -0.5
NCORES = 8
SCHRAUD_A = 184.6650390625   # 128 * log2(e)
SCHRAUD_B0 = 16256.0
SCHRAUD_ADJ = -7.4

# effective engine times (ns) for greedy routing
ACT_T1024 = (1024 + 222) / 1.2
DVE_T1024 = (1024 + 120) / 0.96
DVE_T512 = (512 + 120) / 0.96

_NC_CACHE = {}


TD_SKEW = 250.0
K_FIRST = 8
D_END_BONUS = 0.0
N_WARMUP = 0
OUT_BF16 = True
FIRST_SMALL = 2
FORCE_D_AT = -1
D_PARITY0 = 1


EXP_LAG = 1
PV_LAG = 4


def _build_nc(stage=6):
    import concourse.bass as bass  # noqa: F401
    import concourse.tile as tile
    from concourse import bacc, mybir

    from concourse.alu_op_type import AluOpType
    bf = mybir.dt.bfloat16
    f32 = mybir.dt.float32
    i16 = mybir.dt.int16
    fp8 = mybir.dt.float8e4
    EXP = mybir.ActivationFunctionType.Exp
    OUT_DT_M = bf if OUT_BF16 else f32
    DR = mybir.MatmulPerfMode.DoubleRow

    nc = bacc.Bacc("TRN2", target_bir_lowering=False, debug=False,
                   num_devices=NCORES)

    # q: [65, (j2, t8, var2, i2, 512)]  k: [65, (j2, c32, i2, 128)]
    q_d = nc.declare_dram_parameter("q", [65, 32768], fp8, isOutput=False)
    k_d = nc.declare_dram_parameter("k", [65, 16384], fp8, isOutput=False)
    v_d = nc.declare_dram_parameter("v", [128, 32 * 2 * 33], bf, isOutput=False)
    out_d = nc.declare_dram_parameter("out", [128, 8 * 264], OUT_DT_M, isOutput=True)

    NT = N // 512        # 8 q-tiles of 512
    NCP = 16             # 16 chunk-pairs of 2x128 k rows per (h, t)

    # step order: for t, for h, for p; chunks stream 2 per step
    steps = [(t, h, p) for t in range(NT) for h in range(2) for p in range(NCP)]
    n_steps = len(steps)
    # chunk stream: global chunk g = 2*s + j covers (t, h, c=2p+j)
    n_chunks = 2 * n_steps

    def chunk_info(g):
        t, h, p = steps[g // 2]
        c = 2 * p + (g % 2)
        same = (c // 4 == t)
        return t, h, c, same

    # --- greedy exp-tile schedule over the 128-col quarter stream ---------
    # units (in quarters of 128 cols): ACT tile = 8 (spa, ring-2);
    # DVE alternates 9 (spd1 [1152]) and 4 (spd2 [512]).
    # tiles[i] = (engine, pool_id, q0, nq)
    n_q = 4 * n_chunks
    tiles = []
    ta, td = 0.0, TD_SKEW  # startup skew: DVE's first tile lands later
    d_parity = D_PARITY0
    qq = 0
    COPY_A = (264 + 222) / 1.2
    COPY_D = (264 + 120) / 0.96
    copy_eng = []          # engine per out-copy (t order)
    next_copy_q = 256      # after t=0's quarters (64 chunks * 4 per t)

    def a_cost_of(w):
        return (w + 222) / 1.2

    def d_cost_of(w):
        return (w + 120) / 0.96

    while qq < n_q:
        if qq >= next_copy_q:
            if ta + COPY_A <= td + COPY_D:
                copy_eng.append(0)
                ta += COPY_A
            else:
                copy_eng.append(1)
                td += COPY_D
            next_copy_q += 256
        # first few units are half-size so each engine's first exp can
        # start as soon as a single chunk's scores land
        a_nq = min(4 if len(tiles) < FIRST_SMALL else 8, n_q - qq)
        d_nq = min((4 if len(tiles) < FIRST_SMALL else 8)
                   if d_parity == 0 else 4, n_q - qq)
        d_pool = 1 if d_parity == 0 else 2
        force_d = (len(tiles) == FORCE_D_AT)
        end_bonus = D_END_BONUS if qq > n_q - 420 else 0.0
        if not force_d and \
                ta + a_cost_of(128 * a_nq) <= \
                td + d_cost_of(128 * d_nq) - end_bonus:
            tiles.append(("A", 0, qq, a_nq))
            ta += a_cost_of(128 * a_nq)
            qq += a_nq
        else:
            tiles.append(("D", d_pool, qq, d_nq))
            td += d_cost_of(128 * d_nq)
            d_parity ^= 1
            qq += d_nq
    copy_eng.append(0 if ta <= td else 1)  # final t's copy

    # map: quarter -> (tile_idx, offset_in_tile_in_quarters)
    quarter_loc = {}
    for ti, (_, _, q0, nq) in enumerate(tiles):
        for o in range(nq):
            quarter_loc[q0 + o] = (ti, o)
    # tile of the last quarter of step s (exp(s) ready once this tile done)
    tile_of_step = [quarter_loc[8 * s + 7][0] for s in range(n_steps)]

    with tile.TileContext(nc) as tc:
        from contextlib import ExitStack

        with ExitStack() as ctx:
            const = ctx.enter_context(tc.tile_pool(name="const", bufs=1))

            # dim1 = (j*8 + t)*2 + var  /  j*32 + c
            q_sb = const.tile([65, 32, 2, 512], fp8, tag="q_sb")
            k_sb = const.tile([65, 64, 2, 128], fp8, tag="k_sb")
            v_sb = const.tile([128, 32 * 2 * 33], bf, tag="v_sb")

            def q_ap(j, t, var):
                return q_sb[:, (j * 8 + t) * 2 + var]

            # staged input DMAs: first tiles' operands land early.
            # chunks 0-3 of (h0,t0) are same-class -> var0 slice first.
            kf = K_FIRST
            nc.sync.dma_start(k_sb[:, 0:kf], k_d[:, 0:256 * kf])   # h0 first
            nc.sync.dma_start(q_sb[:, 0:1], q_d[:, 0:1024])        # h0 t0 var0
            nc.sync.dma_start(q_sb[:, 1:2], q_d[:, 1024:2048])     # h0 t0 var1
            nc.sync.dma_start(k_sb[:, kf:32], k_d[:, 256 * kf:8192])  # h0 rest
            nc.sync.dma_start(v_sb[:, 0:528], v_d[:, 0:528])
            nc.sync.dma_start(k_sb[:, 32:64], k_d[:, 8192:16384])  # h1
            nc.sync.dma_start(q_sb[:, 16:18], q_d[:, 16384:18432])  # h1 t0
            nc.sync.dma_start(v_sb[:, 528:2112], v_d[:, 528:2112])
            nc.sync.dma_start(q_sb[:, 2:16], q_d[:, 2048:16384])
            nc.sync.dma_start(q_sb[:, 18:32], q_d[:, 18432:32768])

            # PSUM: ACT ring 2x[1024] + DVE [1024]+[512] + PV [264]
            spa = ctx.enter_context(
                tc.tile_pool(name="spa", bufs=2, space="PSUM"))
            spd1 = ctx.enter_context(
                tc.tile_pool(name="spd1", bufs=1, space="PSUM"))
            spd2 = ctx.enter_context(
                tc.tile_pool(name="spd2", bufs=1, space="PSUM"))
            pvp = ctx.enter_context(
                tc.tile_pool(name="pvp", bufs=1, space="PSUM"))
            ptpa = ctx.enter_context(tc.tile_pool(name="ptpa", bufs=5))
            ptp1 = ctx.enter_context(tc.tile_pool(name="ptp1", bufs=4))
            ptp2 = ctx.enter_context(tc.tile_pool(name="ptp2", bufs=3))
            osp = ctx.enter_context(tc.tile_pool(name="osp", bufs=2))

            # PE p-state warmup: the ramp clock starts at the first PE
            # instruction, so a few dummy matmuls on zeroed scratch during
            # the input-DMA wait make the first real score matmuls run at
            # the mid/full p-state instead of cold.
            if N_WARMUP > 0:
                scr_k = const.tile([65, 2, 128], fp8, tag="scr_k")
                scr_q = const.tile([65, 2, 512], fp8, tag="scr_q")
                nc.gpsimd.memset(scr_k[:], 0)
                nc.gpsimd.memset(scr_q[:], 0)
                for wi in range(N_WARMUP):
                    wsp = spa.tile([128, 1024], f32, tag="sp0",
                                   name=f"warm{wi}")
                    nc.tensor.matmul(wsp[:, 0:512], lhsT=scr_k[:],
                                     rhs=scr_q[:], start=True, stop=True,
                                     perf_mode=DR)

            sp_tiles = {}   # tile_idx -> psum tile
            pt_tiles = {}   # tile_idx -> pt AP (bf16 view)
            pv_tiles = {}   # t -> pv psum tile

            def emit_scores_tile(ti):
                eng, pool_id, q0, nq = tiles[ti]
                w = 128 * nq
                pool = spa if pool_id == 0 else (spd1 if pool_id == 1 else spd2)
                wal = 128 * nq if pool_id == 0 else (1024 if pool_id == 1 else 512)
                sp = pool.tile([128, wal], f32, tag=f"sp{pool_id}",
                               name=f"sp{ti}")
                sp_tiles[ti] = sp
                # one matmul per contiguous quarter-run within a chunk
                q = q0
                while q < q0 + nq:
                    ch = q // 4
                    qe = min((ch + 1) * 4, q0 + nq)
                    cnt = qe - q
                    t, h, c, same = chunk_info(ch)
                    var = 0 if same else 1
                    qc0 = q % 4
                    nc.tensor.matmul(
                        sp[:, (q - q0) * 128:(qe - q0) * 128],
                        lhsT=k_sb[:, h * 32 + c],
                        rhs=q_ap(h, t, var)[:, :, qc0 * 128:
                                            (qc0 + cnt) * 128],
                        start=True, stop=True, perf_mode=DR)
                    q = qe

            def emit_exp_tile(ti):
                eng, pool_id, q0, nq = tiles[ti]
                w = 128 * nq
                sp = sp_tiles.pop(ti)
                if eng == "A":
                    pt = ptpa.tile([128, 1024], bf, tag="pt", name=f"pt{ti}")
                    nc.scalar.activation(
                        pt[:, 0:w], sp[:, 0:w], EXP, bias=0.0, scale=1.0)
                    pt_tiles[ti] = pt[:]
                else:
                    pool, wal = (ptp1, 1024) if pool_id == 1 else (ptp2, 512)
                    pt = pool.tile([128, wal], i16, tag=f"pti{pool_id}",
                                   name=f"pte{ti}")
                    nc.vector.tensor_scalar(
                        pt[:, 0:w], sp[:, 0:w], SCHRAUD_A,
                        SCHRAUD_B0 + SCHRAUD_ADJ,
                        AluOpType.mult, AluOpType.add)
                    pt_tiles[ti] = pt[:].bitcast(bf)

            def emit_pv_step(s):
                t, h, p = steps[s]
                if h == 0 and p == 0:
                    pv_tiles[t] = pvp.tile([128, 264], f32, tag="pv",
                                           name=f"pv{t}")
                pv = pv_tiles[t]
                for j in range(2):
                    c = 2 * p + j
                    for qc in range(4):
                        ti, o = quarter_loc[8 * s + 4 * j + qc]
                        src = pt_tiles[ti]
                        first = (h == 0 and c == 0 and qc == 0)
                        nc.tensor.matmul(
                            pv[:, (h * 4 + qc) * 33:(h * 4 + qc + 1) * 33],
                            lhsT=src[:, o * 128:(o + 1) * 128],
                            rhs=v_sb[:, (c * 2 + h) * 33:(c * 2 + h + 1) * 33],
                            start=first, stop=(c == 31),
                            skip_group_check=True)

            def emit_out(t):
                pv = pv_tiles.pop(t)
                ot = osp.tile([128, 264], OUT_DT_M, tag="ot", name=f"ot{t}")
                if copy_eng[t] == 0:
                    nc.scalar.copy(ot[:], pv[:])
                else:
                    nc.vector.tensor_copy(ot[:], pv[:])
                nc.sync.dma_start(out_d[:, t * 264:(t + 1) * 264], ot[:])

            # software pipeline over steps: scores stream per tile; exp fires
            # one step after a tile's last chunk; PV lags 4 steps.
            next_tile = 0        # next score tile to emit
            exp_done = -1        # last exp-emitted tile
            for s in range(n_steps + 6):
                # emit score tiles covering chunks of step s
                while next_tile < len(tiles) and \
                        tiles[next_tile][2] <= 8 * s + 7 and s < n_steps:
                    emit_scores_tile(next_tile)
                    next_tile += 1
                if 0 <= s - PV_LAG < n_steps:
                    emit_pv_step(s - PV_LAG)
                    # free pt tiles fully consumed (all chunks of tiles
                    # belonging to steps <= s-4 and not needed later)
                if 0 <= s - EXP_LAG < n_steps:
                    # exp for all tiles completed by step s-EXP_LAG
                    target = tile_of_step[s - EXP_LAG]
                    while exp_done < target:
                        exp_done += 1
                        emit_exp_tile(exp_done)
                so = s - 5
                if 0 <= so < n_steps:
                    t, h, p = steps[so]
                    if h == 1 and p == NCP - 1:
                        emit_out(t)

    nc.compile()
    return nc


def _rope(x, positions):
    # x: [..., N, hd]; partial RoPE (rope_percent=0.5)
    half = HD // 2
    ra = half // 2
    frac = 2.0 * np.arange(ra, dtype=np.float32) / HD
    ts = (10000.0 ** frac).astype(np.float32)
    sinu = positions[:, None] / ts[None, :]
    sin = np.sin(sinu).astype(np.float32)
    cos = np.cos(sinu).astype(np.float32)
    f, s = x[..., :half], x[..., half:]
    fr, fp = f[..., :ra], f[..., ra:]
    sr, sp = s[..., :ra], s[..., ra:]
    return np.concatenate(
        [fr * cos - sr * sin, fp, sr * cos + fr * sin, sp], axis=-1)


def _fp8(x):
    return np.asarray(x, dtype=np.float32).astype(FP8)


def kernel(**inputs):
    hs = np.asarray(inputs["hidden_states"], dtype=np.float32)
    qw = np.asarray(inputs["q_w"], dtype=np.float32)
    kw = np.asarray(inputs["k_w"], dtype=np.float32)
    vw = np.asarray(inputs["v_w"], dtype=np.float32)
    ow = np.asarray(inputs["o_w"], dtype=np.float32)
    obb = np.asarray(inputs["o_b"], dtype=np.float32)
    qb_ = np.asarray(inputs["q_b"], dtype=np.float32)
    kb_ = np.asarray(inputs["k_b"], dtype=np.float32)
    vb_ = np.asarray(inputs["v_b"], dtype=np.float32)
    ab = np.asarray(inputs["attention_biases"], dtype=np.float32)
    seq = int(np.asarray(inputs["sequence_length"]))
    assert seq == SEQ, f"kernel compiled for sequence_length={SEQ}, got {seq}"
    assert hs.shape == (B, N, D)

    if ("nc", 6) not in _NC_CACHE:
        _NC_CACHE[("nc", 6)] = _build_nc(6)
    nc = _NC_CACHE[("nc", 6)]

    # host-side projections + rope (f32)
    pos = np.arange(N, dtype=np.float32)
    q = (hs @ qw.T + qb_) * SCALE    # [B, N, D]
    k = hs @ kw.T + kb_
    v = hs @ vw.T + vb_
    q = q.reshape(B, N, H, HD).transpose(0, 2, 1, 3)  # [B, H, N, hd]
    k = k.reshape(B, N, H, HD).transpose(0, 2, 1, 3)
    v = v.reshape(B, N, H, HD).transpose(0, 2, 1, 3)
    q = _rope(q, pos)
    k = _rope(k, pos)

    # fp8 hi/lo factor arrays (shared across cores)
    QH = _fp8(q)
    QHf = QH.astype(np.float32)
    QL8 = _fp8((q - QHf) * 8.0)
    QH4 = _fp8(QHf / 4.0)
    KH = _fp8(k)
    KHf = KH.astype(np.float32)
    KL4 = _fp8((k - KHf) * 4.0)
    KH8 = _fp8(KHf / 8.0)
    KL32 = _fp8(KL4.astype(np.float32) / 32.0)

    in_maps = []
    for c in range(NCORES):
        b = c // 4
        h0 = 2 * (c % 4)
        # q: [65, j, t, var, i, 512]; slice0 rows = [QH(32); QH4(32); bias],
        # slice1 rows = [QL8(32); QL8(32); 0]
        q_t = np.zeros((65, 2, 8, 2, 2, 512), dtype=FP8)
        k_t = np.zeros((65, 2, 32, 2, 128), dtype=FP8)
        v_t = np.empty((128, 32, 2, 33), dtype=np.float32)
        for j in range(2):
            h = h0 + j
            qh = QH[b, h].reshape(8, 512, HD)    # [t, col, hd]
            qh4 = QH4[b, h].reshape(8, 512, HD)
            ql8 = QL8[b, h].reshape(8, 512, HD)
            for var in range(2):
                q_t[0:32, j, :, var, 0] = qh.transpose(2, 0, 1)
                q_t[32:64, j, :, var, 0] = qh4.transpose(2, 0, 1)
                q_t[0:32, j, :, var, 1] = ql8.transpose(2, 0, 1)
                q_t[32:64, j, :, var, 1] = ql8.transpose(2, 0, 1)
                q_t[64, j, :, var, 0] = np.float32(ab[h, var]).astype(FP8)
            kh = KH[b, h].reshape(32, 128, HD)   # [c, col, hd]
            kl4 = KL4[b, h].reshape(32, 128, HD)
            kh8 = KH8[b, h].reshape(32, 128, HD)
            kl32 = KL32[b, h].reshape(32, 128, HD)
            k_t[0:32, j, :, 0] = kh.transpose(2, 0, 1)
            k_t[32:64, j, :, 0] = kl4.transpose(2, 0, 1)
            k_t[64, j, :, 0] = 1.0
            k_t[0:32, j, :, 1] = kh8.transpose(2, 0, 1)
            k_t[32:64, j, :, 1] = kl32.transpose(2, 0, 1)
            v_t[:, :, j, :32] = v[b, h].reshape(32, 128, 32).transpose(1, 0, 2)
            v_t[:, :, j, 32] = 1.0
        in_maps.append({
            "q": np.ascontiguousarray(q_t.reshape(65, 32768)),
            "k": np.ascontiguousarray(k_t.reshape(65, 16384)),
            "v": np.ascontiguousarray(
                v_t.reshape(128, 32 * 2 * 33)).astype(BF16),
        })

    global _LAST_IN_MAPS, _LAST_RESULTS
    _LAST_IN_MAPS = in_maps
    from concourse.bass_utils import run_bass_kernel_spmd
    res = run_bass_kernel_spmd(nc, in_maps, core_ids=list(range(NCORES)))
    _LAST_RESULTS = res.results

    attn = np.empty((B, H, N, HD), dtype=np.float32)
    for c in range(NCORES):
        b = c // 4
        h0 = 2 * (c % 4)
        o = res.results[c]["out"].astype(np.float32).reshape(
            128, 8, 2, 4, 33)
        for j in range(2):
            # q = 512*t + 128*qc + row
            pv = o[:, :, j, :, :32]    # [row, t, qc, 32]
            den = o[:, :, j, :, 32]    # [row, t, qc]
            x = pv / den[..., None]
            attn[b, h0 + j] = x.transpose(1, 2, 0, 3).reshape(N, HD)

    ctx = attn.transpose(0, 2, 1, 3).reshape(B, N, D)
    return ctx @ ow.T + obb[None, None, :]


# revision 48
# speedup vs baseline: 1.0017x; 1.0005x over previous
"""AnyVariateAttention Trainium2 kernel (8 NeuronCores, SPMD).

Sharding: 16 (batch, head) pairs / 8 cores -> core c computes 2 adjacent heads
of batch c//4 (heads 2*(c%4), 2*(c%4)+1).

Host precomputes QKV projections + partial RoPE (cheap O(N*D^2) work) and the
final output projection; the device runs only the O(N^2) attention part.

v3: fp8 DoubleRow score matmuls, bias folded into the matmul, per-engine
PSUM rings, greedy exp routing.  159.9us -> 155.7us (cost-model timeline).

- scores: fp8e4m3 DoubleRow matmuls at 0.5 PE-cycles/row.  Precision comes
  from a hi/lo split: 128 product rows [q_h*k_h | (q_h/4)*(4*k_l) |
  (8*q_l)*(k_h/8) | (8*q_l)*(k_l/8)] + 1 bias row (k side = 1.0, q side =
  the per-(head,class) attention bias) + 1 zero pad = 130 rows = 65
  partitions x 2 DoubleRow slices.  Two q variants carry the same-variate /
  cross-variate bias; the matmul for chunk c of q-tile t picks the variant.
  End-to-end rel err ~6e-3 (vs 2e-2 budget).
- exp is the wall: every score element must leave PSUM through ACT or DVE
  (GPSIMD cannot access PSUM, DMA cannot read PSUM) at 1 elem/lane/cycle.
  With the bias folded into PSUM, exp instructions need no per-class bias
  column, so tiles can group ARBITRARY chunks.  PSUM rings per engine:
  ACT 2x[128,1024], DVE alternating [128,1024]+[128,512], PV [128,264]
  = 15.2KB of the 16KB partition budget.  A greedy list scheduler assigns
  each chunk-group to whichever engine frees up first.
- PV: q in PSUM partitions, out free dim = 33 (head-dim 32 + ones column
  for the softmax denominator), accumulated over 32 k-chunks per q-tile.
- out: unnormalized [pv|den] copied PSUM->SBUF on the less-loaded engine
  and DMAd to DRAM; the host divides by the denominator and applies the
  output projection.
"""

import sys
import numpy as np

for _p in ("/opt/trn_rl_repo",):
    if _p not in sys.path:
        sys.path.insert(0, _p)

import ml_dtypes

BF16 = ml_dtypes.bfloat16
FP8 = ml_dtypes.float8_e4m3

B, N, D, H, HD = 2, 4096, 256, 8, 32
SEQ = 512
SCALE = HD ** -0.5
NCORES = 8
SCHRAUD_A = 184.6650390625   # 128 * log2(e)
SCHRAUD_B0 = 16256.0
SCHRAUD_ADJ = -7.4

# effective engine times (ns) for greedy routing
ACT_T1024 = (1024 + 222) / 1.2
DVE_T1024 = (1024 + 120) / 0.96
DVE_T512 = (512 + 120) / 0.96

_NC_CACHE = {}


TD_SKEW = 250.0
K_FIRST = 8
D_END_BONUS = 0.0
N_WARMUP = 0
OUT_BF16 = True
TAIL_SPLIT = 4
FIRST_SMALL = 2
FORCE_D_AT = -1
D_PARITY0 = 1


EXP_LAG = 1
PV_LAG = 4


def _build_nc(stage=6):
    import concourse.bass as bass  # noqa: F401
    import concourse.tile as tile
    from concourse import bacc, mybir

    from concourse.alu_op_type import AluOpType
    bf = mybir.dt.bfloat16
    f32 = mybir.dt.float32
    i16 = mybir.dt.int16
    fp8 = mybir.dt.float8e4
    EXP = mybir.ActivationFunctionType.Exp
    OUT_DT_M = bf if OUT_BF16 else f32
    DR = mybir.MatmulPerfMode.DoubleRow

    nc = bacc.Bacc("TRN2", target_bir_lowering=False, debug=False,
                   num_devices=NCORES)

    # q: [65, (j2, t8, var2, i2, 512)]  k: [65, (j2, c32, i2, 128)]
    q_d = nc.declare_dram_parameter("q", [65, 32768], fp8, isOutput=False)
    k_d = nc.declare_dram_parameter("k", [65, 16384], fp8, isOutput=False)
    v_d = nc.declare_dram_parameter("v", [128, 32 * 2 * 33], bf, isOutput=False)
    out_d = nc.declare_dram_parameter("out", [128, 8 * 264], OUT_DT_M, isOutput=True)

    NT = N // 512        # 8 q-tiles of 512
    NCP = 16             # 16 chunk-pairs of 2x128 k rows per (h, t)

    # step order: for t, for h, for p; chunks stream 2 per step
    steps = [(t, h, p) for t in range(NT) for h in range(2) for p in range(NCP)]
    n_steps = len(steps)
    # chunk stream: global chunk g = 2*s + j covers (t, h, c=2p+j)
    n_chunks = 2 * n_steps

    def chunk_info(g):
        t, h, p = steps[g // 2]
        c = 2 * p + (g % 2)
        same = (c // 4 == t)
        return t, h, c, same

    # --- greedy exp-tile schedule over the 128-col quarter stream ---------
    # units (in quarters of 128 cols): ACT tile = 8 (spa, ring-2);
    # DVE alternates 9 (spd1 [1152]) and 4 (spd2 [512]).
    # tiles[i] = (engine, pool_id, q0, nq)
    n_q = 4 * n_chunks
    tiles = []
    ta, td = 0.0, TD_SKEW  # startup skew: DVE's first tile lands later
    d_parity = D_PARITY0
    qq = 0
    COPY_A = (264 + 222) / 1.2
    COPY_D = (264 + 120) / 0.96
    copy_eng = []          # engine per out-copy (t order)
    next_copy_q = 256      # after t=0's quarters (64 chunks * 4 per t)

    def a_cost_of(w):
        return (w + 222) / 1.2

    def d_cost_of(w):
        return (w + 120) / 0.96

    while qq < n_q:
        if qq >= next_copy_q:
            if ta + COPY_A <= td + COPY_D:
                copy_eng.append(0)
                ta += COPY_A
            else:
                copy_eng.append(1)
                td += COPY_D
            next_copy_q += 256
        # first few units are half-size so each engine's first exp can
        # start as soon as a single chunk's scores land; the last few are
        # quarter-size so both engines drain the production tail in parallel
        a_cap = 4 if len(tiles) < FIRST_SMALL else 8
        d_cap = (4 if len(tiles) < FIRST_SMALL else 8) if d_parity == 0 else 4
        if n_q - qq <= TAIL_SPLIT:
            a_cap = d_cap = 2
        a_nq = min(a_cap, n_q - qq)
        d_nq = min(d_cap, n_q - qq)
        d_pool = 1 if d_parity == 0 else 2
        force_d = (len(tiles) == FORCE_D_AT)
        end_bonus = D_END_BONUS if qq > n_q - 420 else 0.0
        if not force_d and \
                ta + a_cost_of(128 * a_nq) <= \
                td + d_cost_of(128 * d_nq) - end_bonus:
            tiles.append(("A", 0, qq, a_nq))
            ta += a_cost_of(128 * a_nq)
            qq += a_nq
        else:
            tiles.append(("D", d_pool, qq, d_nq))
            td += d_cost_of(128 * d_nq)
            d_parity ^= 1
            qq += d_nq
    copy_eng.append(0 if ta <= td else 1)  # final t's copy

    # map: quarter -> (tile_idx, offset_in_tile_in_quarters)
    quarter_loc = {}
    for ti, (_, _, q0, nq) in enumerate(tiles):
        for o in range(nq):
            quarter_loc[q0 + o] = (ti, o)
    # tile of the last quarter of step s (exp(s) ready once this tile done)
    tile_of_step = [quarter_loc[8 * s + 7][0] for s in range(n_steps)]

    with tile.TileContext(nc) as tc:
        from contextlib import ExitStack

        with ExitStack() as ctx:
            const = ctx.enter_context(tc.tile_pool(name="const", bufs=1))

            # dim1 = (j*8 + t)*2 + var  /  j*32 + c
            q_sb = const.tile([65, 32, 2, 512], fp8, tag="q_sb")
            k_sb = const.tile([65, 64, 2, 128], fp8, tag="k_sb")
            v_sb = const.tile([128, 32 * 2 * 33], bf, tag="v_sb")

            def q_ap(j, t, var):
                return q_sb[:, (j * 8 + t) * 2 + var]

            # staged input DMAs: first tiles' operands land early.
            # chunks 0-3 of (h0,t0) are same-class -> var0 slice first.
            kf = K_FIRST
            nc.sync.dma_start(k_sb[:, 0:kf], k_d[:, 0:256 * kf])   # h0 first
            nc.sync.dma_start(q_sb[:, 0:1], q_d[:, 0:1024])        # h0 t0 var0
            nc.sync.dma_start(q_sb[:, 1:2], q_d[:, 1024:2048])     # h0 t0 var1
            nc.sync.dma_start(k_sb[:, kf:32], k_d[:, 256 * kf:8192])  # h0 rest
            nc.sync.dma_start(v_sb[:, 0:528], v_d[:, 0:528])
            nc.sync.dma_start(k_sb[:, 32:64], k_d[:, 8192:16384])  # h1
            nc.sync.dma_start(q_sb[:, 16:18], q_d[:, 16384:18432])  # h1 t0
            nc.sync.dma_start(v_sb[:, 528:2112], v_d[:, 528:2112])
            nc.sync.dma_start(q_sb[:, 2:16], q_d[:, 2048:16384])
            nc.sync.dma_start(q_sb[:, 18:32], q_d[:, 18432:32768])

            # PSUM: ACT ring 2x[1024] + DVE [1024]+[512] + PV [264]
            spa = ctx.enter_context(
                tc.tile_pool(name="spa", bufs=2, space="PSUM"))
            spd1 = ctx.enter_context(
                tc.tile_pool(name="spd1", bufs=1, space="PSUM"))
            spd2 = ctx.enter_context(
                tc.tile_pool(name="spd2", bufs=1, space="PSUM"))
            pvp = ctx.enter_context(
                tc.tile_pool(name="pvp", bufs=1, space="PSUM"))
            ptpa = ctx.enter_context(tc.tile_pool(name="ptpa", bufs=5))
            ptp1 = ctx.enter_context(tc.tile_pool(name="ptp1", bufs=4))
            ptp2 = ctx.enter_context(tc.tile_pool(name="ptp2", bufs=3))
            osp = ctx.enter_context(tc.tile_pool(name="osp", bufs=2))

            # PE p-state warmup: the ramp clock starts at the first PE
            # instruction, so a few dummy matmuls on zeroed scratch during
            # the input-DMA wait make the first real score matmuls run at
            # the mid/full p-state instead of cold.
            if N_WARMUP > 0:
                scr_k = const.tile([65, 2, 128], fp8, tag="scr_k")
                scr_q = const.tile([65, 2, 512], fp8, tag="scr_q")
                nc.gpsimd.memset(scr_k[:], 0)
                nc.gpsimd.memset(scr_q[:], 0)
                for wi in range(N_WARMUP):
                    wsp = spa.tile([128, 1024], f32, tag="sp0",
                                   name=f"warm{wi}")
                    nc.tensor.matmul(wsp[:, 0:512], lhsT=scr_k[:],
                                     rhs=scr_q[:], start=True, stop=True,
                                     perf_mode=DR)

            sp_tiles = {}   # tile_idx -> psum tile
            pt_tiles = {}   # tile_idx -> pt AP (bf16 view)
            pv_tiles = {}   # t -> pv psum tile

            def emit_scores_tile(ti):
                eng, pool_id, q0, nq = tiles[ti]
                w = 128 * nq
                pool = spa if pool_id == 0 else (spd1 if pool_id == 1 else spd2)
                wal = 128 * nq if pool_id == 0 else (1024 if pool_id == 1 else 512)
                sp = pool.tile([128, wal], f32, tag=f"sp{pool_id}",
                               name=f"sp{ti}")
                sp_tiles[ti] = sp
                # one matmul per contiguous quarter-run within a chunk
                q = q0
                while q < q0 + nq:
                    ch = q // 4
                    qe = min((ch + 1) * 4, q0 + nq)
                    cnt = qe - q
                    t, h, c, same = chunk_info(ch)
                    var = 0 if same else 1
                    qc0 = q % 4
                    nc.tensor.matmul(
                        sp[:, (q - q0) * 128:(qe - q0) * 128],
                        lhsT=k_sb[:, h * 32 + c],
                        rhs=q_ap(h, t, var)[:, :, qc0 * 128:
                                            (qc0 + cnt) * 128],
                        start=True, stop=True, perf_mode=DR)
                    q = qe

            def emit_exp_tile(ti):
                eng, pool_id, q0, nq = tiles[ti]
                w = 128 * nq
                sp = sp_tiles.pop(ti)
                if eng == "A":
                    pt = ptpa.tile([128, 1024], bf, tag="pt", name=f"pt{ti}")
                    nc.scalar.activation(
                        pt[:, 0:w], sp[:, 0:w], EXP, bias=0.0, scale=1.0)
                    pt_tiles[ti] = pt[:]
                else:
                    pool, wal = (ptp1, 1024) if pool_id == 1 else (ptp2, 512)
                    pt = pool.tile([128, wal], i16, tag=f"pti{pool_id}",
                                   name=f"pte{ti}")
                    nc.vector.tensor_scalar(
                        pt[:, 0:w], sp[:, 0:w], SCHRAUD_A,
                        SCHRAUD_B0 + SCHRAUD_ADJ,
                        AluOpType.mult, AluOpType.add)
                    pt_tiles[ti] = pt[:].bitcast(bf)

            def emit_pv_step(s):
                t, h, p = steps[s]
                if h == 0 and p == 0:
                    pv_tiles[t] = pvp.tile([128, 264], f32, tag="pv",
                                           name=f"pv{t}")
                pv = pv_tiles[t]
                for j in range(2):
                    c = 2 * p + j
                    for qc in range(4):
                        ti, o = quarter_loc[8 * s + 4 * j + qc]
                        src = pt_tiles[ti]
                        first = (h == 0 and c == 0 and qc == 0)
                        nc.tensor.matmul(
                            pv[:, (h * 4 + qc) * 33:(h * 4 + qc + 1) * 33],
                            lhsT=src[:, o * 128:(o + 1) * 128],
                            rhs=v_sb[:, (c * 2 + h) * 33:(c * 2 + h + 1) * 33],
                            start=first, stop=(c == 31),
                            skip_group_check=True)

            def emit_out(t):
                pv = pv_tiles.pop(t)
                ot = osp.tile([128, 264], OUT_DT_M, tag="ot", name=f"ot{t}")
                if copy_eng[t] == 0:
                    nc.scalar.copy(ot[:], pv[:])
                else:
                    nc.vector.tensor_copy(ot[:], pv[:])
                nc.sync.dma_start(out_d[:, t * 264:(t + 1) * 264], ot[:])

            # software pipeline over steps: scores stream per tile; exp fires
            # one step after a tile's last chunk; PV lags 4 steps.
            next_tile = 0        # next score tile to emit
            exp_done = -1        # last exp-emitted tile
            for s in range(n_steps + 6):
                # emit score tiles covering chunks of step s
                while next_tile < len(tiles) and \
                        tiles[next_tile][2] <= 8 * s + 7 and s < n_steps:
                    emit_scores_tile(next_tile)
                    next_tile += 1
                if 0 <= s - PV_LAG < n_steps:
                    emit_pv_step(s - PV_LAG)
                    # free pt tiles fully consumed (all chunks of tiles
                    # belonging to steps <= s-4 and not needed later)
                if 0 <= s - EXP_LAG < n_steps:
                    # exp for all tiles completed by step s-EXP_LAG
                    target = tile_of_step[s - EXP_LAG]
                    while exp_done < target:
                        exp_done += 1
                        emit_exp_tile(exp_done)
                so = s - 5
                if 0 <= so < n_steps:
                    t, h, p = steps[so]
                    if h == 1 and p == NCP - 1:
                        emit_out(t)

    nc.compile()
    return nc


def _rope(x, positions):
    # x: [..., N, hd]; partial RoPE (rope_percent=0.5)
    half = HD // 2
    ra = half // 2
    frac = 2.0 * np.arange(ra, dtype=np.float32) / HD
    ts = (10000.0 ** frac).astype(np.float32)
    sinu = positions[:, None] / ts[None, :]
    sin = np.sin(sinu).astype(np.float32)
    cos = np.cos(sinu).astype(np.float32)
    f, s = x[..., :half], x[..., half:]
    fr, fp = f[..., :ra], f[..., ra:]
    sr, sp = s[..., :ra], s[..., ra:]
    return np.concatenate(
        [fr * cos - sr * sin, fp, sr * cos + fr * sin, sp], axis=-1)


def _fp8(x):
    return np.asarray(x, dtype=np.float32).astype(FP8)


def kernel(**inputs):
    hs = np.asarray(inputs["hidden_states"], dtype=np.float32)
    qw = np.asarray(inputs["q_w"], dtype=np.float32)
    kw = np.asarray(inputs["k_w"], dtype=np.float32)
    vw = np.asarray(inputs["v_w"], dtype=np.float32)
    ow = np.asarray(inputs["o_w"], dtype=np.float32)
    obb = np.asarray(inputs["o_b"], dtype=np.float32)
    qb_ = np.asarray(inputs["q_b"], dtype=np.float32)
    kb_ = np.asarray(inputs["k_b"], dtype=np.float32)
    vb_ = np.asarray(inputs["v_b"], dtype=np.float32)
    ab = np.asarray(inputs["attention_biases"], dtype=np.float32)
    seq = int(np.asarray(inputs["sequence_length"]))
    assert seq == SEQ, f"kernel compiled for sequence_length={SEQ}, got {seq}"
    assert hs.shape == (B, N, D)

    if ("nc", 6) not in _NC_CACHE:
        _NC_CACHE[("nc", 6)] = _build_nc(6)
    nc = _NC_CACHE[("nc", 6)]

    # host-side projections + rope (f32)
    pos = np.arange(N, dtype=np.float32)
    q = (hs @ qw.T + qb_) * SCALE    # [B, N, D]
    k = hs @ kw.T + kb_
    v = hs @ vw.T + vb_
    q = q.reshape(B, N, H, HD).transpose(0, 2, 1, 3)  # [B, H, N, hd]
    k = k.reshape(B, N, H, HD).transpose(0, 2, 1, 3)
    v = v.reshape(B, N, H, HD).transpose(0, 2, 1, 3)
    q = _rope(q, pos)
    k = _rope(k, pos)

    # fp8 hi/lo factor arrays (shared across cores)
    QH = _fp8(q)
    QHf = QH.astype(np.float32)
    QL8 = _fp8((q - QHf) * 8.0)
    QH4 = _fp8(QHf / 4.0)
    KH = _fp8(k)
    KHf = KH.astype(np.float32)
    KL4 = _fp8((k - KHf) * 4.0)
    KH8 = _fp8(KHf / 8.0)
    KL32 = _fp8(KL4.astype(np.float32) / 32.0)

    in_maps = []
    for c in range(NCORES):
        b = c // 4
        h0 = 2 * (c % 4)
        # q: [65, j, t, var, i, 512]; slice0 rows = [QH(32); QH4(32); bias],
        # slice1 rows = [QL8(32); QL8(32); 0]
        q_t = np.zeros((65, 2, 8, 2, 2, 512), dtype=FP8)
        k_t = np.zeros((65, 2, 32, 2, 128), dtype=FP8)
        v_t = np.empty((128, 32, 2, 33), dtype=np.float32)
        for j in range(2):
            h = h0 + j
            qh = QH[b, h].reshape(8, 512, HD)    # [t, col, hd]
            qh4 = QH4[b, h].reshape(8, 512, HD)
            ql8 = QL8[b, h].reshape(8, 512, HD)
            for var in range(2):
                q_t[0:32, j, :, var, 0] = qh.transpose(2, 0, 1)
                q_t[32:64, j, :, var, 0] = qh4.transpose(2, 0, 1)
                q_t[0:32, j, :, var, 1] = ql8.transpose(2, 0, 1)
                q_t[32:64, j, :, var, 1] = ql8.transpose(2, 0, 1)
                q_t[64, j, :, var, 0] = np.float32(ab[h, var]).astype(FP8)
            kh = KH[b, h].reshape(32, 128, HD)   # [c, col, hd]
            kl4 = KL4[b, h].reshape(32, 128, HD)
            kh8 = KH8[b, h].reshape(32, 128, HD)
            kl32 = KL32[b, h].reshape(32, 128, HD)
            k_t[0:32, j, :, 0] = kh.transpose(2, 0, 1)
            k_t[32:64, j, :, 0] = kl4.transpose(2, 0, 1)
            k_t[64, j, :, 0] = 1.0
            k_t[0:32, j, :, 1] = kh8.transpose(2, 0, 1)
            k_t[32:64, j, :, 1] = kl32.transpose(2, 0, 1)
            v_t[:, :, j, :32] = v[b, h].reshape(32, 128, 32).transpose(1, 0, 2)
            v_t[:, :, j, 32] = 1.0
        in_maps.append({
            "q": np.ascontiguousarray(q_t.reshape(65, 32768)),
            "k": np.ascontiguousarray(k_t.reshape(65, 16384)),
            "v": np.ascontiguousarray(
                v_t.reshape(128, 32 * 2 * 33)).astype(BF16),
        })

    global _LAST_IN_MAPS, _LAST_RESULTS
    _LAST_IN_MAPS = in_maps
    from concourse.bass_utils import run_bass_kernel_spmd
    res = run_bass_kernel_spmd(nc, in_maps, core_ids=list(range(NCORES)))
    _LAST_RESULTS = res.results

    attn = np.empty((B, H, N, HD), dtype=np.float32)
    for c in range(NCORES):
        b = c // 4
        h0 = 2 * (c % 4)
        o = res.results[c]["out"].astype(np.float32).reshape(
            128, 8, 2, 4, 33)
        for j in range(2):
            # q = 512*t + 128*qc + row
            pv = o[:, :, j, :, :32]    # [row, t, qc, 32]
            den = o[:, :, j, :, 32]    # [row, t, qc]
            x = pv / den[..., None]
            attn[b, h0 + j] = x.transpose(1, 2, 0, 3).reshape(N, HD)

    ctx = attn.transpose(0, 2, 1, 3).reshape(B, N, D)
    return ctx @ ow.T + obb[None, None, :]
